# revision 22
# baseline (speedup 1.0000x reference)
"""Trainium2 Bass kernel for the GAT pointer-network decoder (CVRP greedy rollout).

Sharding: pure data parallel — batch 1024 split as 128 samples per core on
8 cores; the sample index lives on the SBUF partition dimension. All
parameters are replicated. The n_steps greedy decode loop runs fully
on-device; per-sample attention contractions are Vector-engine ops with
broadcast access patterns (no shared matmul operand exists inside the loop).

Layouts (per core, b = 128 samples on partitions):
  K   [b, (h, n, d)]   glimpse keys
  V   [b, (h, d, n)]   glimpse values
  K2o [b, (n, i)]      pointer keys pre-multiplied by Wo (u = glimpse . K2o)
  G   DRAM [(b n), i]  enc @ fc_w[:, :H].T — gathered by index each step
Only Exp is used on ACT inside the loop (tanh via exp identity); per-step
softmax denominators are stored and Ln runs once in the epilogue.
"""

import numpy as np

import concourse.bacc as bacc
import concourse.bass as bass
import concourse.mybir as mybir
import concourse.tile as tile
from concourse.bass import IndirectOffsetOnAxis
from concourse.bass_utils import run_bass_kernel_spmd
from concourse.masks import make_identity

F32 = mybir.dt.float32
I32 = mybir.dt.int32
U32 = mybir.dt.uint32
AX = mybir.AxisListType
OP = mybir.AluOpType
AF = mybir.ActivationFunctionType

N_CORES = 8
B, N, H = 1024, 100, 128
NH, HD = 8, 16
BC = B // N_CORES
CLIP = 10.0
NEG = -1e9
INV_SQRT_HD = 1.0 / float(np.sqrt(np.float32(HD)))
INV_SQRT_H = 1.0 / float(np.sqrt(np.float32(H)))
PCH = 25          # pointer-logit chunk (nodes per DVE op)

_cache = {}


def _build(n_steps: int, base_cap: float, inv_T: float, no_gather=False, host_iota=False, trunc=99):
    nc = bacc.Bacc("TRN2")

    # packed inputs: one DMA each so no instruction needs two queue sems
    enc_d = nc.dram_tensor("enc", [BC * N, H], F32, kind="ExternalInput")
    wall_d = nc.dram_tensor("wall", [H, 898], F32, kind="ExternalInput")
    bsml_d = nc.dram_tensor("bsml", [BC, H + 1 + N + (1 + N if host_iota else 0)], F32, kind="ExternalInput")
    act_d = nc.dram_tensor("actions", [BC, n_steps], I32, kind="ExternalOutput")
    lp_d = nc.dram_tensor("log_p", [BC, 1], F32, kind="ExternalOutput")

    with tile.TileContext(nc) as tc:
        with (
            tc.tile_pool(name="big", bufs=1) as big,
            tc.tile_pool(name="state", bufs=1) as st,
            tc.tile_pool(name="wts", bufs=1) as wts,
            tc.tile_pool(name="pre", bufs=3) as pre,
            tc.tile_pool(name="ps", bufs=3, space="PSUM") as pp,
            tc.tile_pool(name="ps2", bufs=4, space="PSUM") as pp2,
            tc.tile_pool(name="dram", bufs=1, space="DRAM") as dp,
        ):
            # persistent SBUF
            Ksb = big.tile([BC, NH * N * HD], F32)
            Vsb = big.tile([BC, NH * HD * N], F32)
            K2sb = big.tile([BC, N * H], F32)
            G_dram = dp.tile([BC * N, H], F32)

            mask1 = st.tile([BC, N], F32)
            maskneg = st.tile([BC, N], F32)
            onehot = st.tile([BC, N], F32)
            iotaN = st.tile([BC, N], F32)
            u = st.tile([BC, N], F32)
            udot = st.tile([BC, N], F32)
            mcol = st.tile([BC, N], F32)     # scratch for mask rebuild
            cap = st.tile([BC, 1], F32)
            cnt = st.tile([BC, 1], F32)
            seld = st.tile([BC, 1], F32)
            done = st.tile([BC, 1], F32)
            ung = st.tile([BC, 1], F32)
            allc = st.tile([BC, 1], F32)
            s1 = st.tile([BC, 1], F32)
            idxf = st.tile([BC, 1], F32)
            idxl = st.tile([BC, 1], I32)
            ci = st.tile([BC, 1], I32)
            base_t = st.tile([BC, 1], F32)
            usel = st.tile([BC, 1], F32)
            tmp1 = st.tile([BC, 1], F32)
            lsum = st.tile([BC, 1], F32)
            top8 = st.tile([BC, 8], F32)
            idx8 = st.tile([BC, 8], U32)
            sums = st.tile([BC, NH], F32)
            rsum = st.tile([BC, NH], F32)
            scores = st.tile([BC, NH * N], F32)   # becomes attn (in-place exp)
            glimpse = st.tile([BC, H], F32)
            graw = st.tile([BC, H], F32)
            dec = st.tile([BC, H], F32)
            decT = st.tile([H, BC], F32)
            ggath = st.tile([BC, H], F32)
            pproj = st.tile([BC, H], F32)
            fccol = st.tile([BC, H], F32)
            cf = st.tile([BC, H], F32)
            acts = st.tile([BC, n_steps], I32)
            ssteps = st.tile([BC, n_steps], F32)
            prod = st.tile([BC, 2 * N * HD], F32)  # 3200 f32 shared scratch
            ident = st.tile([128, 128], F32)

            wsb = wts.tile([H, 898], F32)
            nc.sync.dma_start(wsb[:], wall_d[:])
            w_nat = {
                "fcw": wsb[:, 0:129], "fc1": wsb[:, 129:257],
                "wq": wsb[:, 257:385], "wk": wsb[:, 385:513],
                "wv": wsb[:, 513:641], "wo": wsb[:, 641:769],
                "wk2": wsb[:, 769:897], "ones": wsb[:, 897:898],
            }
            bsb = wts.tile([BC, H + 1 + N + (1 + N if host_iota else 0)], F32)
            nc.sync.dma_start(bsb[:], bsml_d[:])
            dem = bsb[:, H + 1:H + 1 + N]
            make_identity(nc, ident[:])
            # dummy PE op: absorbs the gpsimd wait so later matmuls carry <=1 wait
            dmy = pp.tile([128, 128], F32, tag="ps")
            nc.tensor.transpose(dmy[:], ident[:], ident[:])

            def transpose_to(dst_ap, src_ap):
                ps = pp.tile([128, 128], F32, tag="ps")
                nc.tensor.transpose(ps[:src_ap.shape[1], :src_ap.shape[0]],
                                    src_ap, ident[:])
                nc.scalar.copy(dst_ap, ps[:dst_ap.shape[0], :dst_ap.shape[1]])

            RqT = wts.tile([H, H], F32)
            RkT = wts.tile([H, H], F32)
            RvT = wts.tile([H, H], F32)
            RfcT = wts.tile([H, H], F32)
            M2 = wts.tile([H, H], F32)
            transpose_to(RqT[:], w_nat["wq"])
            transpose_to(RkT[:], w_nat["wk"])
            transpose_to(RvT[:], w_nat["wv"])
            transpose_to(RfcT[:], w_nat["fcw"][:, :H])
            nc.vector.tensor_scalar_mul(RqT[:], RqT[:], INV_SQRT_HD)

            m2ps = pp.tile([128, 128], F32, tag="ps")
            nc.tensor.matmul(m2ps[:], lhsT=w_nat["wk2"], rhs=w_nat["wo"])
            nc.scalar.copy(M2[:], m2ps[:])

            # fc_w's capacity column replicated across partitions via k=1 matmul
            fcol1 = pre.tile([1, H], F32, tag="one")
            transpose_to(fcol1[:1, :], w_nat["fcw"][:, H:H + 1])
            ones1 = pre.tile([1, BC], F32, tag="one1")
            transpose_to(ones1[:1, :], w_nat["ones"])
            fcps = pp.tile([BC, H], F32, tag="ps")
            nc.tensor.matmul(fcps[:], lhsT=ones1[:1, :], rhs=fcol1[:1, :])
            nc.scalar.copy(fccol[:], fcps[:])

            # pool_proj = pool @ fc1.T  ==  (fc1 @ pool.T).T
            poolsb = bsb[:, :H]
            Rfc1T = pre.tile([H, H], F32, tag="fc1t")
            transpose_to(Rfc1T[:], w_nat["fc1"])
            pt = pp.tile([128, 128], F32, tag="ps")
            nc.tensor.transpose(pt[:], poolsb, ident[:])
            poolT = pre.tile([H, BC], F32, tag="poolT")
            nc.scalar.copy(poolT[:], pt[:])
            ppp = pp.tile([128, 128], F32, tag="ps")
            nc.tensor.matmul(ppp[:], lhsT=poolT[:], rhs=Rfc1T[:])
            nc.scalar.copy(pproj[:], ppp[:])

            # encoder projections
            enc_v = enc_d[:].rearrange("(b n) i -> b n i", n=N)
            g_v = G_dram[:].rearrange("(b n) i -> b n i", n=N)
            K4 = Ksb[:].rearrange("b (h n d) -> b h n d", h=NH, n=N)
            V4 = Vsb[:].rearrange("b (h d n) -> b h d n", h=NH, d=HD)
            for n in range(N):
                encn = pre.tile([BC, H], F32, tag="encn")
                nc.sync.dma_start(encn[:], enc_v[:, n, :])
                etp = pp.tile([128, 128], F32, tag="ps")
                nc.tensor.transpose(etp[:], encn[:], ident[:])
                encT = pre.tile([H, BC], F32, tag="encT")
                nc.scalar.copy(encT[:], etp[:])
                kps = pp2.tile([BC, H], F32, tag="proj")
                nc.tensor.matmul(kps[:], lhsT=encT[:], rhs=RkT[:])
                nc.vector.tensor_copy(K4[:, :, n, :],
                                      kps[:].rearrange("b (h d) -> b h d", h=NH))
                vps = pp2.tile([BC, H], F32, tag="proj")
                nc.tensor.matmul(vps[:], lhsT=encT[:], rhs=RvT[:])
                nc.vector.tensor_copy(V4[:, :, :, n],
                                      vps[:].rearrange("b (h d) -> b h d", h=NH))
                k2ps = pp2.tile([BC, H], F32, tag="proj")
                nc.tensor.matmul(k2ps[:], lhsT=encT[:], rhs=M2[:])
                nc.scalar.copy(K2sb[:, n * H:(n + 1) * H], k2ps[:])
                gps = pp2.tile([BC, H], F32, tag="proj")
                nc.tensor.matmul(gps[:], lhsT=encT[:], rhs=RfcT[:])
                gsb = pre.tile([BC, H], F32, tag="gsb")
                nc.scalar.copy(gsb[:], gps[:])
                nc.sync.dma_start(g_v[:, n, :], gsb[:])

            # initial state
            nc.vector.tensor_copy(cap[:], bsb[:, H:H + 1])
            nc.sync.dma_start(ggath[:], g_v[:, 0, :])
            if host_iota:
                nc.vector.tensor_copy(ci[:], bsb[:, H + 1 + N:H + 2 + N])
                nc.vector.tensor_copy(iotaN[:], bsb[:, H + 2 + N:])
            else:
                nc.gpsimd.iota(ci[:], pattern=[[0, 1]], base=0, channel_multiplier=N)
                iotaI = pre.tile([BC, N], I32, tag="encn")
                nc.gpsimd.iota(iotaI[:], pattern=[[1, N]], base=0, channel_multiplier=0)
                nc.vector.tensor_copy(iotaN[:], iotaI[:])
            nc.vector.memset(base_t[:], float(base_cap))
            nc.vector.memset(mask1[:], 0.0)
            nc.vector.memset(cnt[:], 0.0)
            nc.vector.memset(usel[:], 0.0)
            nc.vector.tensor_tensor(mcol[:], dem, cap[:].to_broadcast([BC, N]),
                                    op=OP.is_gt)
            nc.vector.tensor_reduce(allc[:], mcol[:, 1:], axis=AX.X, op=OP.min)
            nc.vector.tensor_scalar(mcol[:, 0:1], allc[:], -1.0, 1.0,
                                    op0=OP.mult, op1=OP.add)
            nc.vector.tensor_scalar_mul(maskneg[:], mcol[:], NEG)

            # views for the loop
            K5 = Ksb[:].rearrange("b (p x n d) -> b p x n d", p=NH // 2, x=2, n=N)
            V5 = Vsb[:].rearrange("b (p x d n) -> b p x d n", p=NH // 2, x=2, d=HD)
            K2v = K2sb[:].rearrange("b (n i) -> b n i", n=N)
            sc3 = scores[:].rearrange("b (p n) -> b p n", p=NH // 2)
            at5 = scores[:].rearrange("b (p x n) -> b p x n", p=NH // 2, x=2)
            g4 = graw[:].rearrange("b (p x d) -> b p x d", p=NH // 2, x=2)
            prod4k = prod[:].rearrange("b (x n d) -> b x n d", x=2, n=N)
            prod4v = prod[:].rearrange("b (x d n) -> b x d n", x=2, d=HD)
            prod3p = prod[:, :PCH * H].rearrange("b (n i) -> b n i", i=H)

            for t in range(n_steps):
                if trunc < 1:
                    continue
                # dec = G[idx] + pool_proj + cap * fc_col
                nc.vector.tensor_tensor(dec[:], ggath[:], pproj[:], op=OP.add)
                nc.vector.tensor_scalar(cf[:], fccol[:], cap[:], None, op0=OP.mult)
                nc.vector.tensor_tensor(dec[:], dec[:], cf[:], op=OP.add)
                # q = dec @ (Wq/sqrt(hd)).T  on PE
                dtp = pp.tile([128, 128], F32, tag="ps")
                nc.tensor.transpose(dtp[:], dec[:], ident[:])
                nc.scalar.copy(decT[:], dtp[:])
                qps = pp.tile([BC, H], F32, tag="ps")
                nc.tensor.matmul(qps[:], lhsT=decT[:], rhs=RqT[:])
                q4 = qps[:].rearrange("b (p x d) -> b p x d", p=NH // 2, x=2)

                if trunc < 2:
                    continue
                # scores[b,h,n] = sum_d q*K  (two heads per op)
                for p in range(NH // 2):
                    qb = q4[:, p].unsqueeze(2).to_broadcast([BC, 2, N, HD])
                    nc.vector.tensor_tensor(prod4k, K5[:, p], qb, op=OP.mult)
                    nc.vector.tensor_reduce(
                        sc3[:, p].rearrange("b (x n) -> b x n", x=2),
                        prod4k, axis=AX.X, op=OP.add)
                mb = maskneg[:].unsqueeze(1).to_broadcast([BC, NH, N])
                nc.vector.tensor_tensor(
                    scores[:].rearrange("b (h n) -> b h n", h=NH),
                    scores[:].rearrange("b (h n) -> b h n", h=NH), mb, op=OP.add)
                # softmax numerator + per-head sums (max-sub safe to skip)
                for h in range(NH):
                    nc.scalar.activation(scores[:, h * N:(h + 1) * N],
                                         scores[:, h * N:(h + 1) * N], AF.Exp,
                                         accum_out=sums[:, h:h + 1])
                nc.vector.reciprocal(rsum[:], sums[:])
                if trunc < 3:
                    continue
                # glimpse_raw[b,h,d] = sum_n attn*V
                for p in range(NH // 2):
                    ab = at5[:, p].unsqueeze(2).to_broadcast([BC, 2, HD, N])
                    nc.vector.tensor_tensor(prod4v, V5[:, p], ab, op=OP.mult)
                    nc.vector.tensor_reduce(g4[:, p], prod4v, axis=AX.X, op=OP.add)
                rb = rsum[:].unsqueeze(2).to_broadcast([BC, NH, HD])
                nc.vector.tensor_tensor(
                    glimpse[:].rearrange("b (h d) -> b h d", h=NH),
                    graw[:].rearrange("b (h d) -> b h d", h=NH), rb, op=OP.mult)
                # pointer logits
                gb = glimpse[:].unsqueeze(1).to_broadcast([BC, PCH, H])
                for c in range(N // PCH):
                    nc.vector.tensor_tensor(prod3p, K2v[:, c * PCH:(c + 1) * PCH],
                                            gb, op=OP.mult)
                    nc.vector.tensor_reduce(udot[:, c * PCH:(c + 1) * PCH],
                                            prod3p, axis=AX.X, op=OP.add)
                if trunc < 4:
                    continue
                # u = CLIP*tanh(udot/sqrt(H))/T + maskNEG ; tanh via exp
                nc.scalar.activation(u[:], udot[:], AF.Exp, scale=2.0 * INV_SQRT_H)
                nc.vector.tensor_scalar(u[:], u[:], 1.0, None, op0=OP.add)
                nc.vector.reciprocal(u[:], u[:])
                nc.vector.tensor_scalar(u[:], u[:], -2.0 * CLIP * inv_T,
                                        CLIP * inv_T, op0=OP.mult, op1=OP.add)
                nc.vector.tensor_tensor(u[:], u[:], maskneg[:], op=OP.add)
                if trunc < 5:
                    continue
                # greedy argmax
                nc.vector.max(top8[:], u[:])
                nc.vector.max_index(idx8[:], top8[:], u[:])
                if trunc < 6:
                    continue
                # gate from pre-update visited count
                nc.vector.tensor_scalar(done[:], cnt[:], float(N - 1), None,
                                        op0=OP.is_ge)
                nc.vector.tensor_scalar(ung[:], done[:], -1.0, 1.0,
                                        op0=OP.mult, op1=OP.add)
                # softmax denominator for log_p (ln deferred)
                nc.scalar.activation(prod[:, :N], u[:], AF.Exp, accum_out=s1[:])
                nc.vector.scalar_tensor_tensor(ssteps[:, t:t + 1], s1[:], ung[:],
                                               done[:], op0=OP.mult, op1=OP.add)
                nc.vector.scalar_tensor_tensor(usel[:], top8[:, 0:1], ung[:],
                                               usel[:], op0=OP.mult, op1=OP.add)
                if trunc < 7:
                    continue
                # one-hot; demand; capacity
                nc.vector.tensor_copy(idxf[:], idx8[:, 0:1])
                nc.vector.tensor_tensor(onehot[:], iotaN[:],
                                        idxf[:].to_broadcast([BC, N]), op=OP.is_equal)
                nc.vector.tensor_tensor(prod[:, :N], dem, onehot[:], op=OP.mult)
                nc.vector.tensor_reduce(seld[:], prod[:, :N], axis=AX.X, op=OP.add)
                nc.vector.tensor_tensor(cap[:], cap[:], seld[:], op=OP.subtract)
                nc.vector.tensor_tensor(tmp1[:], base_t[:], cap[:], op=OP.subtract)
                nc.vector.scalar_tensor_tensor(cap[:], tmp1[:], onehot[:, 0:1],
                                               cap[:], op0=OP.mult, op1=OP.add)
                if trunc < 8:
                    continue
                # visited set, feasibility, depot rule
                nc.vector.tensor_tensor(mask1[:], mask1[:], onehot[:], op=OP.max)
                nc.vector.memset(mask1[:, 0:1], 0.0)
                nc.vector.tensor_reduce(cnt[:], mask1[:, 1:], axis=AX.X, op=OP.add)
                nc.vector.tensor_tensor(mcol[:], dem,
                                        cap[:].to_broadcast([BC, N]), op=OP.is_gt)
                nc.vector.tensor_tensor(mcol[:], mcol[:], mask1[:], op=OP.max)
                nc.vector.tensor_reduce(allc[:], mcol[:, 1:], axis=AX.X, op=OP.min)
                nc.vector.tensor_scalar(allc[:], allc[:], 0.0, None, op0=OP.is_gt)
                nc.vector.tensor_scalar(allc[:], allc[:], -1.0, 1.0,
                                        op0=OP.mult, op1=OP.add)
                nc.vector.tensor_tensor(mcol[:, 0:1], onehot[:, 0:1], allc[:],
                                        op=OP.mult)
                nc.vector.tensor_scalar_mul(maskneg[:], mcol[:], NEG)
                if trunc < 9:
                    continue
                # record action; gather next embedding row G[b, idx]
                nc.vector.tensor_copy(acts[:, t:t + 1], idx8[:, 0:1].bitcast(I32))
                nc.vector.tensor_tensor(idxl[:], ci[:], idx8[:, 0:1].bitcast(I32),
                                        op=OP.add)
                if no_gather:
                    nc.sync.dma_start(ggath[:], g_v[:, 0, :])
                else:
                    nc.gpsimd.indirect_dma_start(
                        out=ggath[:], out_offset=None, in_=G_dram[:],
                        in_offset=IndirectOffsetOnAxis(ap=idxl[:, :1], axis=0))

            # epilogue
            if trunc >= 10:
                nc.scalar.activation(ssteps[:], ssteps[:], AF.Ln, accum_out=lsum[:])
                nc.vector.tensor_tensor(s1[:], usel[:], lsum[:], op=OP.subtract)
                nc.sync.dma_start(lp_d[:], s1[:])
                nc.sync.dma_start(act_d[:], acts[:])
            else:
                nc.vector.memset(s1[:], 0.0)
                nc.vector.memset(acts[:], 0)
                nc.sync.dma_start(lp_d[:], s1[:])
                nc.sync.dma_start(act_d[:], acts[:])

    nc.finalize()
    return nc


def kernel(encoder_inputs, pool, capacity, demand, fc_w, fc1_w, Wq, Wk, Wv, Wo,
           Wk2, n_steps, T, greedy):
    n_steps = int(np.asarray(n_steps))
    T = float(np.asarray(T))
    assert int(np.asarray(greedy)) == 1
    enc = np.ascontiguousarray(np.asarray(encoder_inputs, np.float32))
    pool = np.ascontiguousarray(np.asarray(pool, np.float32))
    capacity = np.ascontiguousarray(np.asarray(capacity, np.float32))
    demand = np.ascontiguousarray(np.asarray(demand, np.float32))
    base_cap = float(capacity[0, 0])

    key = (n_steps, base_cap, 1.0 / T)
    if key not in _cache:
        _cache[key] = _build(n_steps, base_cap, 1.0 / T)
    nc = _cache[key]

    wall = np.concatenate(
        [np.asarray(a, np.float32) for a in (fc_w, fc1_w, Wq, Wk, Wv, Wo, Wk2)]
        + [np.ones((H, 1), np.float32)], axis=1)
    wall = np.ascontiguousarray(wall)
    bsml = np.ascontiguousarray(
        np.concatenate([pool, capacity, demand], axis=1).astype(np.float32))
    in_maps = []
    for c in range(N_CORES):
        s = slice(c * BC, (c + 1) * BC)
        in_maps.append({
            "enc": np.ascontiguousarray(enc[s].reshape(BC * N, H)),
            "wall": wall,
            "bsml": bsml[s],
        })
    res = run_bass_kernel_spmd(nc, in_maps, list(range(N_CORES)))
    outs = res.results
    actions = np.concatenate([np.asarray(o["actions"]) for o in outs], axis=0)
    log_p = np.concatenate([np.asarray(o["log_p"]).reshape(BC) for o in outs], axis=0)
    return actions.astype(np.int32), log_p.astype(np.float32)


# revision 24
# speedup vs baseline: 1.1386x; 1.1386x over previous
"""Trainium2 Bass kernel for the GAT pointer-network decoder (CVRP greedy rollout).

Sharding: pure data parallel — batch 1024 split as 128 samples per core on
8 cores; the sample index lives on the SBUF partition dimension. All
parameters are replicated. The n_steps greedy decode loop runs fully
on-device; per-sample attention contractions are Vector-engine ops with
broadcast access patterns (no shared matmul operand exists inside the loop).

Layouts (per core, b = 128 samples on partitions):
  K   [b, (h, n, d)]   glimpse keys
  V   [b, (h, d, n)]   glimpse values
  K2o [b, (n, i)]      pointer keys pre-multiplied by Wo (u = glimpse . K2o)
  G   DRAM [(b n), i]  enc @ fc_w[:, :H].T — gathered by index each step
Only Exp is used on ACT inside the loop (tanh via exp identity); per-step
softmax denominators are stored and Ln runs once in the epilogue.
"""

import numpy as np

import concourse.bacc as bacc
import concourse.bass as bass
import concourse.mybir as mybir
import concourse.tile as tile
from concourse.bass import IndirectOffsetOnAxis
from concourse.bass_utils import run_bass_kernel_spmd
from concourse.masks import make_identity

F32 = mybir.dt.float32
I32 = mybir.dt.int32
U32 = mybir.dt.uint32
AX = mybir.AxisListType
OP = mybir.AluOpType
AF = mybir.ActivationFunctionType

N_CORES = 8
B, N, H = 1024, 100, 128
NH, HD = 8, 16
BC = B // N_CORES
CLIP = 10.0
NEG = -1e9
INV_SQRT_HD = 1.0 / float(np.sqrt(np.float32(HD)))
INV_SQRT_H = 1.0 / float(np.sqrt(np.float32(H)))
PCH = 25          # pointer-logit chunk (nodes per DVE op)

_cache = {}


def _build(n_steps: int, base_cap: float, inv_T: float, no_gather=False, host_iota=False):
    nc = bacc.Bacc("TRN2")

    # packed inputs: one DMA each so no instruction needs two queue sems
    enc_d = nc.dram_tensor("enc", [BC * N, H], F32, kind="ExternalInput")
    wall_d = nc.dram_tensor("wall", [H, 898], F32, kind="ExternalInput")
    bsml_d = nc.dram_tensor("bsml", [BC, H + 1 + N + (1 + N if host_iota else 0)], F32, kind="ExternalInput")
    act_d = nc.dram_tensor("actions", [BC, n_steps], I32, kind="ExternalOutput")
    lp_d = nc.dram_tensor("log_p", [BC, 1], F32, kind="ExternalOutput")

    with tile.TileContext(nc) as tc:
        with (
            tc.tile_pool(name="big", bufs=1) as big,
            tc.tile_pool(name="state", bufs=1) as st,
            tc.tile_pool(name="wts", bufs=1) as wts,
            tc.tile_pool(name="pre", bufs=3) as pre,
            tc.tile_pool(name="ps", bufs=3, space="PSUM") as pp,
            tc.tile_pool(name="ps2", bufs=4, space="PSUM") as pp2,
            tc.tile_pool(name="dram", bufs=1, space="DRAM") as dp,
        ):
            # persistent SBUF
            Ksb = big.tile([BC, NH * N * HD], F32)
            Vsb = big.tile([BC, NH * HD * N], F32)
            K2sb = big.tile([BC, N * H], F32)
            G_dram = dp.tile([BC * N, H + 4], F32)  # +demand col (+pad)

            mask1 = st.tile([BC, N], F32)
            maskneg = st.tile([BC, N], F32)
            onehot = st.tile([BC, N], F32)
            iotaN = st.tile([BC, N], F32)
            u = st.tile([BC, N], F32)
            udot = st.tile([BC, N], F32)
            mcol = st.tile([BC, N], F32)     # scratch for mask rebuild
            cap = st.tile([BC, 1], F32)
            cnt = st.tile([BC, 1], F32)
            seld = st.tile([BC, 1], F32)
            done = st.tile([BC, 1], F32)
            ung = st.tile([BC, 1], F32)
            allc = st.tile([BC, 1], F32)
            s1 = st.tile([BC, 1], F32)
            idxf = st.tile([BC, 1], F32)
            idxl = st.tile([BC, 1], I32)
            ci = st.tile([BC, 1], I32)
            base_t = st.tile([BC, 1], F32)
            usel = st.tile([BC, 1], F32)
            tmp1 = st.tile([BC, 1], F32)
            notd = st.tile([BC, 1], F32)
            lsum = st.tile([BC, 1], F32)
            top8 = st.tile([BC, 8], F32)
            idx8 = st.tile([BC, 8], U32)
            sums = st.tile([BC, NH], F32)
            rsum = st.tile([BC, NH], F32)
            scores = st.tile([BC, NH * N], F32)   # becomes attn (in-place exp)
            glimpse = st.tile([BC, H], F32)
            graw = st.tile([BC, H], F32)
            dec = st.tile([BC, H], F32)
            decT = st.tile([H, BC], F32)
            q_sb = st.tile([BC, H], F32)
            ggath = st.tile([BC, H + 4], F32)
            pproj = st.tile([BC, H], F32)
            fccol = st.tile([BC, H], F32)
            cf = st.tile([BC, H], F32)
            acts = st.tile([BC, n_steps], I32)
            ssteps = st.tile([BC, n_steps], F32)
            prod = st.tile([BC, 2 * N * HD], F32)  # 3200 f32 shared scratch
            prod_g = st.tile([BC, 2 * N * HD], F32)  # gpsimd-side scratch
            ident = st.tile([128, 128], F32)

            wsb = wts.tile([H, 898], F32)
            nc.sync.dma_start(wsb[:], wall_d[:])
            w_nat = {
                "fcw": wsb[:, 0:129], "fc1": wsb[:, 129:257],
                "wq": wsb[:, 257:385], "wk": wsb[:, 385:513],
                "wv": wsb[:, 513:641], "wo": wsb[:, 641:769],
                "wk2": wsb[:, 769:897], "ones": wsb[:, 897:898],
            }
            bsb = wts.tile([BC, H + 1 + N + (1 + N if host_iota else 0)], F32)
            nc.sync.dma_start(bsb[:], bsml_d[:])
            dem = bsb[:, H + 1:H + 1 + N]
            make_identity(nc, ident[:])
            # dummy PE op: absorbs the gpsimd wait so later matmuls carry <=1 wait
            dmy = pp.tile([128, 128], F32, tag="ps")
            nc.tensor.transpose(dmy[:], ident[:], ident[:])

            def transpose_to(dst_ap, src_ap):
                ps = pp.tile([128, 128], F32, tag="ps")
                nc.tensor.transpose(ps[:src_ap.shape[1], :src_ap.shape[0]],
                                    src_ap, ident[:])
                nc.scalar.copy(dst_ap, ps[:dst_ap.shape[0], :dst_ap.shape[1]])

            RqT = wts.tile([H, H], F32)
            RkT = wts.tile([H, H], F32)
            RvT = wts.tile([H, H], F32)
            RfcT = wts.tile([H, H], F32)
            M2 = wts.tile([H, H], F32)
            transpose_to(RqT[:], w_nat["wq"])
            transpose_to(RkT[:], w_nat["wk"])
            transpose_to(RvT[:], w_nat["wv"])
            transpose_to(RfcT[:], w_nat["fcw"][:, :H])
            nc.vector.tensor_scalar_mul(RqT[:], RqT[:], INV_SQRT_HD)

            m2ps = pp.tile([128, 128], F32, tag="ps")
            nc.tensor.matmul(m2ps[:], lhsT=w_nat["wk2"], rhs=w_nat["wo"])
            nc.scalar.copy(M2[:], m2ps[:])

            # fc_w's capacity column replicated across partitions via k=1 matmul
            fcol1 = pre.tile([1, H], F32, tag="one")
            transpose_to(fcol1[:1, :], w_nat["fcw"][:, H:H + 1])
            ones1 = pre.tile([1, BC], F32, tag="one1")
            transpose_to(ones1[:1, :], w_nat["ones"])
            fcps = pp.tile([BC, H], F32, tag="ps")
            nc.tensor.matmul(fcps[:], lhsT=ones1[:1, :], rhs=fcol1[:1, :])
            nc.scalar.copy(fccol[:], fcps[:])

            # pool_proj = pool @ fc1.T  ==  (fc1 @ pool.T).T
            poolsb = bsb[:, :H]
            Rfc1T = pre.tile([H, H], F32, tag="fc1t")
            transpose_to(Rfc1T[:], w_nat["fc1"])
            pt = pp.tile([128, 128], F32, tag="ps")
            nc.tensor.transpose(pt[:], poolsb, ident[:])
            poolT = pre.tile([H, BC], F32, tag="poolT")
            nc.scalar.copy(poolT[:], pt[:])
            ppp = pp.tile([128, 128], F32, tag="ps")
            nc.tensor.matmul(ppp[:], lhsT=poolT[:], rhs=Rfc1T[:])
            nc.scalar.copy(pproj[:], ppp[:])

            # encoder projections
            enc_v = enc_d[:].rearrange("(b n) i -> b n i", n=N)
            g_v = G_dram[:].rearrange("(b n) i -> b n i", n=N)
            K4 = Ksb[:].rearrange("b (h n d) -> b h n d", h=NH, n=N)
            V4 = Vsb[:].rearrange("b (h d n) -> b h d n", h=NH, d=HD)
            for n in range(N):
                encn = pre.tile([BC, H], F32, tag="encn")
                nc.sync.dma_start(encn[:], enc_v[:, n, :])
                etp = pp.tile([128, 128], F32, tag="ps")
                nc.tensor.transpose(etp[:], encn[:], ident[:])
                encT = pre.tile([H, BC], F32, tag="encT")
                nc.scalar.copy(encT[:], etp[:])
                kps = pp2.tile([BC, H], F32, tag="proj")
                nc.tensor.matmul(kps[:], lhsT=encT[:], rhs=RkT[:])
                nc.vector.tensor_copy(K4[:, :, n, :],
                                      kps[:].rearrange("b (h d) -> b h d", h=NH))
                vps = pp2.tile([BC, H], F32, tag="proj")
                nc.tensor.matmul(vps[:], lhsT=encT[:], rhs=RvT[:])
                nc.vector.tensor_copy(V4[:, :, :, n],
                                      vps[:].rearrange("b (h d) -> b h d", h=NH))
                k2ps = pp2.tile([BC, H], F32, tag="proj")
                nc.tensor.matmul(k2ps[:], lhsT=encT[:], rhs=M2[:])
                nc.scalar.copy(K2sb[:, n * H:(n + 1) * H], k2ps[:])
                gps = pp2.tile([BC, H], F32, tag="proj")
                nc.tensor.matmul(gps[:], lhsT=encT[:], rhs=RfcT[:])
                gsb = pre.tile([BC, H], F32, tag="gsb")
                nc.scalar.copy(gsb[:], gps[:])
                nc.sync.dma_start(g_v[:, n, :H], gsb[:])

            # demand column of the gather table (sel_d arrives with the row)
            nc.sync.dma_start(g_v[:, :, H:H + 1], dem.unsqueeze(2))

            # initial state
            nc.vector.tensor_copy(cap[:], bsb[:, H:H + 1])
            nc.sync.dma_start(ggath[:], g_v[:, 0, :])
            if host_iota:
                nc.vector.tensor_copy(ci[:], bsb[:, H + 1 + N:H + 2 + N])
                nc.vector.tensor_copy(iotaN[:], bsb[:, H + 2 + N:])
            else:
                nc.gpsimd.iota(ci[:], pattern=[[0, 1]], base=0, channel_multiplier=N)
                iotaI = pre.tile([BC, N], I32, tag="encn")
                nc.gpsimd.iota(iotaI[:], pattern=[[1, N]], base=0, channel_multiplier=0)
                nc.vector.tensor_copy(iotaN[:], iotaI[:])
            nc.vector.memset(base_t[:], float(base_cap))
            nc.vector.memset(mask1[:], 0.0)
            nc.vector.memset(cnt[:], 0.0)
            nc.vector.memset(usel[:], 0.0)
            nc.vector.tensor_tensor(mcol[:], dem, cap[:].to_broadcast([BC, N]),
                                    op=OP.is_gt)
            nc.vector.tensor_reduce(allc[:], mcol[:, 1:], axis=AX.X, op=OP.min)
            nc.vector.tensor_scalar(mcol[:, 0:1], allc[:], -1.0, 1.0,
                                    op0=OP.mult, op1=OP.add)
            nc.vector.tensor_scalar_mul(maskneg[:], mcol[:], NEG)

            # views for the loop
            K5 = Ksb[:].rearrange("b (p x n d) -> b p x n d", p=NH // 2, x=2, n=N)
            V5 = Vsb[:].rearrange("b (p x d n) -> b p x d n", p=NH // 2, x=2, d=HD)
            K2v = K2sb[:].rearrange("b (n i) -> b n i", n=N)
            sc3 = scores[:].rearrange("b (p n) -> b p n", p=NH // 2)
            at5 = scores[:].rearrange("b (p x n) -> b p x n", p=NH // 2, x=2)
            g4 = graw[:].rearrange("b (p x d) -> b p x d", p=NH // 2, x=2)
            prod4k = prod[:].rearrange("b (x n d) -> b x n d", x=2, n=N)
            prod4v = prod[:].rearrange("b (x d n) -> b x d n", x=2, d=HD)
            prod3p = prod[:, :PCH * H].rearrange("b (n i) -> b n i", i=H)
            prod4g = prod_g[:].rearrange("b (x n d) -> b x n d", x=2, n=N)
            prod4gv = prod_g[:].rearrange("b (x d n) -> b x d n", x=2, d=HD)
            prod3g = prod_g[:, :PCH * H].rearrange("b (n i) -> b n i", i=H)

            for t in range(n_steps):
                # dec = G[idx] + pool_proj + cap * fc_col
                nc.vector.tensor_tensor(dec[:], ggath[:, :H], pproj[:], op=OP.add)
                nc.vector.tensor_scalar(cf[:], fccol[:], cap[:], None, op0=OP.mult)
                nc.vector.tensor_tensor(dec[:], dec[:], cf[:], op=OP.add)
                # q = dec @ (Wq/sqrt(hd)).T  on PE
                dtp = pp.tile([128, 128], F32, tag="ps")
                nc.tensor.transpose(dtp[:], dec[:], ident[:])
                nc.vector.tensor_copy(decT[:], dtp[:])
                qps = pp.tile([BC, H], F32, tag="ps")
                nc.tensor.matmul(qps[:], lhsT=decT[:], rhs=RqT[:])
                nc.vector.tensor_copy(q_sb[:], qps[:])
                q4 = q_sb[:].rearrange("b (p x d) -> b p x d", p=NH // 2, x=2)

                # scores[b,h,n] = sum_d q*K  (two heads per op)
                for p in range(NH // 2):
                    qb = q4[:, p].unsqueeze(2).to_broadcast([BC, 2, N, HD])
                    eng = nc.gpsimd if p == 3 else nc.vector
                    pr = prod4g if p == 3 else prod4k
                    eng.tensor_tensor(pr, K5[:, p], qb, op=OP.mult)
                    nc.vector.tensor_reduce(
                        sc3[:, p].rearrange("b (x n) -> b x n", x=2),
                        pr, axis=AX.X, op=OP.add)
                mb = maskneg[:].unsqueeze(1).to_broadcast([BC, NH, N])
                nc.vector.tensor_tensor(
                    scores[:].rearrange("b (h n) -> b h n", h=NH),
                    scores[:].rearrange("b (h n) -> b h n", h=NH), mb, op=OP.add)
                # softmax numerator + per-head sums (max-sub safe to skip)
                for h in range(NH):
                    nc.scalar.activation(scores[:, h * N:(h + 1) * N],
                                         scores[:, h * N:(h + 1) * N], AF.Exp,
                                         accum_out=sums[:, h:h + 1])
                nc.vector.reciprocal(rsum[:], sums[:])
                # glimpse_raw[b,h,d] = sum_n attn*V
                for p in range(NH // 2):
                    ab = at5[:, p].unsqueeze(2).to_broadcast([BC, 2, HD, N])
                    eng = nc.gpsimd if p == 3 else nc.vector
                    pr = prod4gv if p == 3 else prod4v
                    eng.tensor_tensor(pr, V5[:, p], ab, op=OP.mult)
                    nc.vector.tensor_reduce(g4[:, p], pr, axis=AX.X, op=OP.add)
                rb = rsum[:].unsqueeze(2).to_broadcast([BC, NH, HD])
                nc.vector.tensor_tensor(
                    glimpse[:].rearrange("b (h d) -> b h d", h=NH),
                    graw[:].rearrange("b (h d) -> b h d", h=NH), rb, op=OP.mult)
                # pointer logits
                gb = glimpse[:].unsqueeze(1).to_broadcast([BC, PCH, H])
                for c in range(N // PCH):
                    eng = nc.gpsimd if c == 3 else nc.vector
                    pr = prod3g if c == 3 else prod3p
                    eng.tensor_tensor(pr, K2v[:, c * PCH:(c + 1) * PCH],
                                      gb, op=OP.mult)
                    nc.vector.tensor_reduce(udot[:, c * PCH:(c + 1) * PCH],
                                            pr, axis=AX.X, op=OP.add)
                # u = CLIP*tanh(udot/sqrt(H))/T + maskNEG ; tanh via exp
                nc.scalar.activation(u[:], udot[:], AF.Exp, scale=2.0 * INV_SQRT_H)
                nc.vector.tensor_scalar(u[:], u[:], 1.0, None, op0=OP.add)
                nc.vector.reciprocal(u[:], u[:])
                nc.vector.tensor_scalar(u[:], u[:], -2.0 * CLIP * inv_T,
                                        CLIP * inv_T, op0=OP.mult, op1=OP.add)
                nc.vector.tensor_tensor(u[:], u[:], maskneg[:], op=OP.add)
                # greedy argmax
                nc.vector.max(top8[:], u[:])
                nc.vector.max_index(idx8[:], top8[:], u[:])
                # gate from pre-update visited count
                nc.vector.tensor_scalar(done[:], cnt[:], float(N - 1), None,
                                        op0=OP.is_ge)
                nc.vector.tensor_scalar(ung[:], done[:], -1.0, 1.0,
                                        op0=OP.mult, op1=OP.add)
                # softmax denominator for log_p (ln deferred)
                nc.scalar.activation(prod[:, :N], u[:], AF.Exp, accum_out=s1[:])
                nc.vector.scalar_tensor_tensor(ssteps[:, t:t + 1], s1[:], ung[:],
                                               done[:], op0=OP.mult, op1=OP.add)
                nc.vector.scalar_tensor_tensor(usel[:], top8[:, 0:1], ung[:],
                                               usel[:], op0=OP.mult, op1=OP.add)
                # state updates on GPSIMD (off the DVE critical path);
                # gather FIRST so the cap update reads THIS step's row
                nc.vector.tensor_copy(idxf[:], idx8[:, 0:1])
                nc.vector.tensor_tensor(idxl[:], ci[:], idx8[:, 0:1].bitcast(I32),
                                        op=OP.add)
                nc.vector.tensor_copy(acts[:, t:t + 1], idx8[:, 0:1].bitcast(I32))
                if no_gather:
                    nc.sync.dma_start(ggath[:], g_v[:, 0, :])
                else:
                    nc.gpsimd.indirect_dma_start(
                        out=ggath[:], out_offset=None, in_=G_dram[:],
                        in_offset=IndirectOffsetOnAxis(ap=idxl[:, :1], axis=0))
                nc.vector.tensor_tensor(onehot[:], iotaN[:],
                                        idxf[:].to_broadcast([BC, N]), op=OP.is_equal)
                nc.vector.tensor_tensor(cap[:], cap[:], ggath[:, H:H + 1],
                                        op=OP.subtract)
                nc.vector.tensor_tensor(tmp1[:], base_t[:], cap[:], op=OP.subtract)
                nc.vector.tensor_tensor(tmp1[:], tmp1[:], onehot[:, 0:1], op=OP.mult)
                nc.vector.tensor_tensor(cap[:], cap[:], tmp1[:], op=OP.add)
                # visited set; incremental count (customers are never revisited)
                nc.vector.tensor_tensor(mask1[:], mask1[:], onehot[:], op=OP.max)
                nc.vector.memset(mask1[:, 0:1], 0.0)
                nc.vector.tensor_scalar(notd[:], onehot[:, 0:1], -1.0, 1.0,
                                        op0=OP.mult, op1=OP.add)
                nc.vector.tensor_tensor(cnt[:], cnt[:], notd[:], op=OP.add)
                nc.vector.tensor_tensor(mcol[:], dem,
                                        cap[:].to_broadcast([BC, N]), op=OP.is_gt)
                nc.vector.tensor_tensor(mcol[:], mcol[:], mask1[:], op=OP.max)
                nc.vector.tensor_reduce(allc[:], mcol[:, 1:], axis=AX.X, op=OP.min)
                nc.vector.tensor_scalar(allc[:], allc[:], 0.0, None, op0=OP.is_gt)
                nc.vector.tensor_scalar(allc[:], allc[:], -1.0, 1.0,
                                        op0=OP.mult, op1=OP.add)
                nc.vector.tensor_tensor(mcol[:, 0:1], onehot[:, 0:1], allc[:],
                                        op=OP.mult)
                nc.vector.tensor_scalar_mul(maskneg[:], mcol[:], NEG)

            # epilogue
            nc.scalar.activation(ssteps[:], ssteps[:], AF.Ln, accum_out=lsum[:])
            nc.vector.tensor_tensor(s1[:], usel[:], lsum[:], op=OP.subtract)
            nc.sync.dma_start(lp_d[:], s1[:])
            nc.sync.dma_start(act_d[:], acts[:])

    nc.finalize()
    return nc


def kernel(encoder_inputs, pool, capacity, demand, fc_w, fc1_w, Wq, Wk, Wv, Wo,
           Wk2, n_steps, T, greedy):
    n_steps = int(np.asarray(n_steps))
    T = float(np.asarray(T))
    assert int(np.asarray(greedy)) == 1
    enc = np.ascontiguousarray(np.asarray(encoder_inputs, np.float32))
    pool = np.ascontiguousarray(np.asarray(pool, np.float32))
    capacity = np.ascontiguousarray(np.asarray(capacity, np.float32))
    demand = np.ascontiguousarray(np.asarray(demand, np.float32))
    base_cap = float(capacity[0, 0])

    key = (n_steps, base_cap, 1.0 / T)
    if key not in _cache:
        _cache[key] = _build(n_steps, base_cap, 1.0 / T)
    nc = _cache[key]

    wall = np.concatenate(
        [np.asarray(a, np.float32) for a in (fc_w, fc1_w, Wq, Wk, Wv, Wo, Wk2)]
        + [np.ones((H, 1), np.float32)], axis=1)
    wall = np.ascontiguousarray(wall)
    bsml = np.ascontiguousarray(
        np.concatenate([pool, capacity, demand], axis=1).astype(np.float32))
    in_maps = []
    for c in range(N_CORES):
        s = slice(c * BC, (c + 1) * BC)
        in_maps.append({
            "enc": np.ascontiguousarray(enc[s].reshape(BC * N, H)),
            "wall": wall,
            "bsml": bsml[s],
        })
    res = run_bass_kernel_spmd(nc, in_maps, list(range(N_CORES)))
    outs = res.results
    actions = np.concatenate([np.asarray(o["actions"]) for o in outs], axis=0)
    log_p = np.concatenate([np.asarray(o["log_p"]).reshape(BC) for o in outs], axis=0)
    return actions.astype(np.int32), log_p.astype(np.float32)
